# revision 1
# baseline (speedup 1.0000x reference)
"""Trainium2 Bass kernel for nn_Attention (32-head attention, partial rotary,
dense softmax) sharded 4-heads-per-core across 8 NeuronCores.

Self-contained: takes full unsharded inputs, returns the full output.

Design notes (per core, heads h = 4c..4c+3, N=2048 tokens, d_head=256, e=128):
  - All projections computed in transposed [feature, token] layout.
  - V path eliminated: host precomputes M_h = Wproj_h @ Wv_h so the
    attention value tensor U_h = x @ M_h.T comes from one matmul per
    128-token tile (U_h[j,e] = sum_c x[j,c] M_h[e,c]).
  - rotate_half baked into 64 extra "swapped" stationary columns merged
    into the same projection matmul (output rows 64:128 of the rot bank),
    so rotary = one [128,512] multiply by [cos;sin] + one [64,512] add.
  - d-dim layout per head: slot0 p0:64 = rotated rot dims, slot0 p64:128
    = d 192:256, slot1 = d 64:192.  Scores in fp8-E4M3 DoubleRow.
  - Softmax without max-subtraction (|score*scale| small by construction).
    exp runs on ACT in [128,2,512] two-bank batches; denominator is ONE
    strided DVE bf16 add (16 exp tiles -> 8 pair partials, 2x mode) per
    chunk, shipped raw to the host which finishes the sum in f32.
  - Normalization + head/core reduction + bias on the host.
  - ACT does only exp; PSUM evictions on DVE; GpSimd takes the SBUF-only
    rotary adds so DVE stays under the ACT/PE ceiling.
"""

import sys

sys.path.insert(0, "/opt/trn_rl_repo")

import numpy as np
import ml_dtypes

import concourse.bacc as bacc
import concourse.tile as tile
from concourse import mybir
from concourse.bass_utils import run_bass_kernel_spmd

DIM = 128
HEADS = 32
DH = 256          # per-head dim
ROT = 64          # partial rotary width
N = 2048
NCORES = 8
HPC = HEADS // NCORES  # heads per core = 4
SCALE = float(DIM) ** -0.5

BF16 = mybir.dt.bfloat16
FP8 = mybir.dt.float8e4
F32 = mybir.dt.float32
EXP = mybir.ActivationFunctionType.Exp
DR = mybir.MatmulPerfMode.DoubleRow

BF16_NP = ml_dtypes.bfloat16


def build_nc(n=N):
    """Build the per-core Bass program (identical for all cores; SPMD)."""
    assert n % 512 == 0
    nch = n // 512   # 512-wide query chunks
    njt = n // 128   # 128-wide key tiles
    njp = njt // 2   # key-tile pairs

    nc = bacc.Bacc("TRN2", target_bir_lowering=False, debug=False,
                   num_devices=NCORES)

    xT = nc.dram_tensor("xT", [128, n], BF16, kind="ExternalInput")
    # A-stationary: cols 0:64 rot rows, 64:128 swapped rot rows
    wA = nc.dram_tensor("wA", [128, HPC, 2, 128], BF16, kind="ExternalInput")
    wB = nc.dram_tensor("wB", [128, HPC, 2, 128], BF16, kind="ExternalInput")
    # q-X and k-X projections merged in one stationary (cols 0:64 | 64:128)
    wC = nc.dram_tensor("wC", [128, HPC, 128], BF16, kind="ExternalInput")
    Mw = nc.dram_tensor("Mw", [128, HPC, 128], BF16, kind="ExternalInput")
    trig = nc.dram_tensor("trig", [128, n], F32, kind="ExternalInput")
    uv = nc.dram_tensor("uv", [HPC, 128, n], F32, kind="ExternalOutput")
    # raw exp tiles; host computes the softmax denominators
    den = nc.dram_tensor("den", [HPC, nch, 128, njp, 2, 512], BF16,
                         kind="ExternalOutput")

    with tile.TileContext(nc) as tc:
        with (
            tc.tile_pool(name="consts", bufs=1) as consts,
            tc.tile_pool(name="hd", bufs=2) as hd,
            tc.tile_pool(name="es", bufs=3) as es,
            tc.tile_pool(name="tmp", bufs=4) as tmp,
            tc.tile_pool(name="ps", bufs=2, space="PSUM") as ps,
        ):
            wA_sb = consts.tile([128, HPC, 2, 128], BF16)
            nc.sync.dma_start(out=wA_sb[:, 0, :, :], in_=wA[:, 0, :, :])
            xT_sb = consts.tile([128, n], BF16)
            for ci in range(nch):   # chunked so the first matmul starts early
                sl = slice(ci * 512, ci * 512 + 512)
                nc.sync.dma_start(out=xT_sb[:, sl], in_=xT[:, sl])
            for hh in range(1, HPC):
                nc.sync.dma_start(out=wA_sb[:, hh, :, :], in_=wA[:, hh, :, :])
            trig_sb = consts.tile([128, n], F32)
            nc.gpsimd.dma_start(out=trig_sb, in_=trig[:, :])
            wB_sb = consts.tile([128, HPC, 2, 128], BF16)
            nc.gpsimd.dma_start(out=wB_sb, in_=wB[:, :, :, :])
            wC_sb = consts.tile([128, HPC, 128], BF16)
            nc.gpsimd.dma_start(out=wC_sb, in_=wC[:, :, :])
            M_sb = consts.tile([128, HPC, 128], BF16)
            nc.gpsimd.dma_start(out=M_sb, in_=Mw[:, :, :])

            def alloc_head_tiles():
                qT_sb = hd.tile([128, 2, n], FP8, tag="q", name="qT_sb")
                kT_sb = hd.tile([128, 2, n], FP8, tag="k", name="kT_sb")
                U_sb = hd.tile([128, njt, 128], BF16, tag="u", name="U_sb")
                return qT_sb, kT_sb, U_sb

            def emit_qk_ci(h, t, tiles, ci, cold=False):
                # q (t=0) or k (t=1) projection for one 512-token chunk,
                # fused rotary via the merged rot+swap A matmul.  The q part
                # also carries the merged q-X|k-X projection for both heads'
                # slot0 lower halves.
                qT_sb, kT_sb, _ = tiles
                outT = qT_sb if t == 0 else kT_sb
                sl = slice(ci * 512, ci * 512 + 512)
                psA = ps.tile([128, 512], F32, tag="pp", bufs=2, name="psA")
                nc.tensor.matmul(psA, wA_sb[:, h, t, :], xT_sb[:, sl],
                                 start=True, stop=True)
                # two base-0 SBUF halves (PSUM input may be partition-
                # shifted; two SBUF inputs of an add may not)
                mc = tmp.tile([64, 512], F32, tag="m", bufs=2, name="mc")
                nc.vector.tensor_mul(mc, psA[0:64, :], trig_sb[0:64, sl])
                msw = tmp.tile([64, 512], F32, tag="m2", bufs=2, name="msw")
                nc.vector.tensor_mul(msw, psA[64:128, :],
                                     trig_sb[64:128, sl])
                # rot rows -> slot0 p0:64 (fp8); SBUF-only, so GpSimd
                nc.gpsimd.tensor_add(outT[0:64, 0, sl], mc, msw)
                # during the cold start ACT is idle: use it for the
                # evictions so the serial DVE chain shortens
                cp = nc.scalar.copy if cold else nc.vector.tensor_copy
                psB = ps.tile([128, 512], F32, tag="pp", bufs=2, name="psB")
                nc.tensor.matmul(psB, wB_sb[:, h, t, :], xT_sb[:, sl],
                                 start=True, stop=True)
                cp(outT[:, 1, sl], psB)
                psC = ps.tile([128, 512], F32, tag="pp", bufs=2, name="psC")
                nc.tensor.matmul(psC[64:128, :], wC_sb[:, h, t * 64:
                                                       t * 64 + 64],
                                 xT_sb[:, sl], start=True, stop=True)
                cp(outT[64:128, 0, sl], psC[64:128, :])

            def emit_u4(h, U_sb, jt4):
                # U_h[j, e] = sum_c x[j, c] M_h[c, e]; 4 key-tiles per bank
                psu = ps.tile([128, 4, 128], F32, tag="pp", bufs=2, name="psu")
                for t in range(4):
                    jsl = slice((jt4 + t) * 128, (jt4 + t) * 128 + 128)
                    nc.tensor.matmul(psu[:, t, :], xT_sb[:, jsl],
                                     M_sb[:, h, :], start=True, stop=True)
                nc.vector.tensor_copy(U_sb[:, jt4:jt4 + 4, :], psu)

            def build_parts(h, tiles):
                # k parts first: the first score matmul needs full kT but
                # only chunk 0 of qT
                qT_sb, kT_sb, U_sb = tiles
                cold = h == 0
                parts = []
                for t in (1, 0):
                    for ci in range(nch):
                        parts.append(
                            lambda h=h, t=t, tiles=tiles, ci=ci,
                            cold=(cold and (t == 1 or ci == 0)):
                            emit_qk_ci(h, t, tiles, ci, cold))
                for jt4 in range(0, njt, 4):
                    parts.append(lambda h=h, U_sb=U_sb, jt4=jt4:
                                 emit_u4(h, U_sb, jt4))
                return parts

            def emit_scores_exp(h, ci, tiles, last=False):
                qT_sb, kT_sb, _ = tiles
                isl = slice(ci * 512, ci * 512 + 512)
                expS = es.tile([128, njp, 2, 512], BF16, tag="e")
                # raw exp tiles ship in slices (host computes denominators);
                # the final chunk ships finer so its last transfer is short
                cuts = (4, 6, 8) if last else (4, 8)
                lo = 0
                for jp in range(njp):
                    j0 = slice(jp * 256, jp * 256 + 128)
                    j1 = slice(jp * 256 + 128, jp * 256 + 256)
                    pss = ps.tile([128, 2, 512], F32, tag="pss", bufs=2)
                    nc.tensor.matmul(pss[:, 0, :], kT_sb[:, :, j0],
                                     qT_sb[:, :, isl], start=True,
                                     stop=True, perf_mode=DR)
                    nc.tensor.matmul(pss[:, 1, :], kT_sb[:, :, j1],
                                     qT_sb[:, :, isl], start=True,
                                     stop=True, perf_mode=DR)
                    nc.scalar.activation(expS[:, jp, :, :], pss, EXP,
                                         scale=SCALE)
                    if jp + 1 in cuts:
                        nc.sync.dma_start(out=den[h, ci, :, lo:jp + 1],
                                          in_=expS[:, lo:jp + 1])
                        lo = jp + 1
                return expS

            def emit_uv_tail(h, ci, tiles, expS):
                _, _, U_sb = tiles
                isl = slice(ci * 512, ci * 512 + 512)
                psuv = ps.tile([128, 512], F32, tag="puv", bufs=2)
                for jp in range(njp):
                    nc.tensor.matmul(psuv, U_sb[:, 2 * jp, :],
                                     expS[:, jp, 0, :],
                                     start=(jp == 0), stop=False)
                    nc.tensor.matmul(psuv, U_sb[:, 2 * jp + 1, :],
                                     expS[:, jp, 1, :],
                                     start=False, stop=(jp == njp - 1))
                ouv = tmp.tile([128, 512], F32, tag="ouv", bufs=2)
                nc.vector.tensor_copy(ouv, psuv)
                nc.sync.dma_start(out=uv[h, :, isl], in_=ouv)

            # ---- schedule: chunk g+1's scores/exp are emitted before
            # chunk g's uv block so PE streams scores while ACT drains exp;
            # next head's projections ride between chunks, finishing before
            # their head's first score is emitted.
            tiles = {0: alloc_head_tiles()}
            p0 = build_parts(0, tiles[0])
            # q chunk 0 first (first score needs it), then k chunks in
            # order (score pair jp gates only on its own kT sub-range)
            for part in [p0[4]] + p0[0:4]:
                part()
            due = {g: [] for g in range(nch * HPC)}
            due[0] = p0[5:8] + p0[8:12]     # q1..q3 + all U
            for h in range(HPC - 1):
                tiles[h + 1] = alloc_head_tiles()
                ph = build_parts(h + 1, tiles[h + 1])
                due[nch * h + 1] += ph[0:4]    # k parts
                due[nch * h + 2] += ph[4:8]    # q parts
                due[nch * h + 3] += ph[8:12]   # U parts
            prev = None
            for g in range(nch * HPC):
                h, ci = divmod(g, nch)
                expS = emit_scores_exp(h, ci, tiles[h],
                                       last=(g == nch * HPC - 1))
                if prev is not None:
                    emit_uv_tail(*prev)
                prev = (h, ci, tiles[h], expS)
                for part in due[g]:
                    part()
            emit_uv_tail(*prev)

    nc.compile()
    return nc


# swap within the 64 rot dims: rotate_half pairs (d, d+32)
_PERM = np.concatenate([np.arange(32, 64), np.arange(0, 32)])


def prep_core(core, x, Wqkv, Wproj, rot, n=N):
    """Build the per-core input map (numpy, host-side sharding/layout)."""
    hs = slice(core * HPC, (core + 1) * HPC)
    W4 = Wqkv.reshape(3, HEADS, DH, DIM)

    wA = np.empty((128, HPC, 2, 128), np.float32)
    wB = np.empty((128, HPC, 2, 128), np.float32)
    wC = np.empty((128, HPC, 128), np.float32)
    for t in range(2):
        w = W4[t, hs]                      # [HPC, 256, 128]
        wA[:, :, t, 0:64] = w[:, 0:64].transpose(2, 0, 1)
        wA[:, :, t, 64:128] = w[:, _PERM].transpose(2, 0, 1)
        wB[:, :, t, :] = w[:, 64:192].transpose(2, 0, 1)
        wC[:, :, t * 64:t * 64 + 64] = w[:, 192:256].transpose(2, 0, 1)

    # M_h[c, e] = sum_d Wv_h[d, c] * Wp_h[e, d]
    Wp = Wproj.reshape(DIM, HEADS, DH)[:, hs]      # [128 e, HPC, 256 d]
    Wv = W4[2, hs]                                 # [HPC, 256 d, 128 c]
    Mw = np.einsum("ehd,hdc->che", Wp.transpose(0, 1, 2), Wv,
                   optimize=True).astype(np.float32)

    trig = np.empty((128, n), np.float32)
    trig[0:64] = np.cos(rot).T
    sinT = np.sin(rot).T.copy()
    sinT[:32] *= -1.0   # rotate_half sign for output rows 0:32
    trig[64:128] = sinT

    xT = np.ascontiguousarray(x.reshape(n, DIM).T).astype(BF16_NP)  # [128, n]

    return {
        "xT": xT,
        "wA": np.ascontiguousarray(wA).astype(BF16_NP),
        "wB": np.ascontiguousarray(wB).astype(BF16_NP),
        "wC": np.ascontiguousarray(wC).astype(BF16_NP),
        "Mw": np.ascontiguousarray(Mw).astype(BF16_NP),
        "trig": trig,
    }


def postprocess(results, bproj, n=N):
    """Finish denominators, normalize, sum heads/cores, add bias."""
    acc = np.zeros((DIM, n), np.float64)
    for r in results:
        uvr = np.asarray(r["uv"], np.float64)        # [HPC, 128, n]
        d = r["den"]               # [HPC, nch, 128, njp, 2, 512] bf16
        dsum = np.empty((HPC, n // 512, 512), np.float64)
        for h in range(HPC):
            dsum[h] = np.asarray(d[h], np.float32).sum(
                axis=(1, 2, 3), dtype=np.float64)    # [nch, 512]
        dsum = dsum.reshape(HPC, n)                  # [HPC, n]
        acc += (uvr / dsum[:, None, :]).sum(axis=0)
    out = acc.T + np.asarray(bproj, np.float64)[None, :]
    return out.astype(np.float32).reshape(1, n, DIM)


_NC_CACHE = {}


def _get_nc(n=N):
    if n not in _NC_CACHE:
        _NC_CACHE[n] = build_nc(n)
    return _NC_CACHE[n]


def kernel(x, Wqkv, Wproj, bproj, rotary_pos_emb):
    x = np.asarray(x, np.float32)
    Wqkv = np.asarray(Wqkv, np.float32)
    Wproj = np.asarray(Wproj, np.float32)
    bproj = np.asarray(bproj, np.float32)
    rot = np.asarray(rotary_pos_emb, np.float32)

    nc = _get_nc(N)
    in_maps = [prep_core(c, x, Wqkv, Wproj, rot, N) for c in range(NCORES)]
    res = run_bass_kernel_spmd(nc, in_maps, core_ids=list(range(NCORES)))
    return postprocess(res.results, bproj, N)

